# revision 9
# baseline (speedup 1.0000x reference)
"""KNN top-20 kernel for TRN2 (8 NeuronCores, SPMD).

Problem: x (4, 8192, 16) f32. For each point, indices of the 20 nearest
neighbors (squared L2, self included) among the 8192 points of its batch,
ordered ascending by distance -> output (x, idx[4,8192,20] int32).

Sharding: 8 cores = 4 batches x 2 half-row-shards. Each core handles 4096
query rows against all 8192 candidates of its batch.

Per-core algorithm (per 128-query row-tile, 32 tiles):
  - PE: K=17 fp32 matmul -> PSUM bank [128q, 512c] of the negated-distance
    key  2<x_q, x_c> - |x_c|^2  (row 16 of lhsT is 1.0, of rhs is -|x_c|^2).
    Larger key == closer. The per-row constant |x_q|^2 is dropped (does not
    affect ranking).
  - DVE: per bank `max` (top-8 values, desc) + `max_index` (their in-chunk
    indices). 16 banks -> 128 candidate values/indices per row. Safe unless
    a row has >8 of its top-20 in one 512-chunk (rare; host fixup below).
  - DVE: phase 2 - top-24 of the 128 candidates via 3 x (max+match_replace).
  - GPSIMD: per-winner index extraction: scalar_tensor_tensor
    (cv == w_j) * cand_global_idx, accumulated over the free dim.
  - Host: rows where a 512-chunk saturates the top-20 cutoff, or with exact
    value ties among the top-24, are recomputed exactly on the host.

Engine-dependency discipline: hardware instructions hold only ~2 semaphore
waits, so ops are placed to keep each instruction's distinct cross-engine
dependencies <= 2 (index-side ops live on GPSIMD; export DMAs are chained
after the GPSIMD reads so pool releases collapse to one semaphore).
"""

import numpy as np

B, N, C = 4, 8192, 16
K = 20
ROWS_PER_CORE = 4096
N_TILES_FULL = ROWS_PER_CORE // 128
NEG_BIG = -3.0e38

_NC_CACHE = {}


def build_nc(n_tiles=N_TILES_FULL, psum_direct=True):
    import concourse.bacc as bacc
    import concourse.bass as bass
    import concourse.mybir as mybir
    from concourse.tile import TileContext

    f32 = mybir.dt.float32
    i32 = mybir.dt.int32
    u32 = mybir.dt.uint32
    Alu = mybir.AluOpType

    nc = bacc.Bacc("TRN2", target_bir_lowering=False, debug=False)

    n_rows = n_tiles * 128
    qT = nc.dram_tensor("qT", [17, n_rows], f32, kind="ExternalInput")
    cT = nc.dram_tensor("cT", [17, N], f32, kind="ExternalInput")
    idx_out = nc.dram_tensor("idx_out", [n_rows, K], i32, kind="ExternalOutput")
    w_out = nc.dram_tensor("w_out", [n_rows, 24], f32, kind="ExternalOutput")
    cv_out = nc.dram_tensor("cv_out", [n_rows, 128], f32, kind="ExternalOutput")

    n_banks = N // 512  # 16

    with TileContext(nc) as tc:
        with (
            tc.tile_pool(name="const", bufs=1) as const_pool,
            tc.tile_pool(name="psum", bufs=8, space="PSUM") as psum_pool,
            tc.tile_pool(name="stage", bufs=3) as stage_pool,
            tc.tile_pool(name="work", bufs=3) as work_pool,
        ):
            # raw loads, then DVE-staged copies so downstream consumers wait
            # on a single DVE semaphore instead of N DMA-queue semaphores.
            qT_ld = const_pool.tile([17, n_rows], f32)
            nc.sync.dma_start(qT_ld[:], qT[:])
            cT_ld = const_pool.tile([17, N], f32)
            nc.sync.dma_start(cT_ld[:], cT[:])
            qT_sb = const_pool.tile([17, n_rows], f32)
            nc.vector.tensor_copy(qT_sb[:], qT_ld[:])
            cT_sb = const_pool.tile([17, N], f32)
            nc.vector.tensor_copy(cT_sb[:], cT_ld[:])

            # base[s] = (s >> 3) * 512 for s in 0..127 (chunk base of cand slot)
            base_i = const_pool.tile([128, 128], i32)
            nc.gpsimd.iota(base_i[:], pattern=[[512, 16], [0, 8]], base=0,
                           channel_multiplier=0)
            base_f = const_pool.tile([128, 128], f32)
            nc.gpsimd.tensor_copy(base_f[:], base_i[:])

            for t in range(n_tiles):
                cv = work_pool.tile([128, 128], f32, tag="cv")
                ci = work_pool.tile([128, 128], u32, tag="ci")
                for b in range(n_banks):
                    ps = psum_pool.tile([128, 512], f32, tag="ps")
                    nc.tensor.matmul(
                        ps[:],
                        qT_sb[:, t * 128:(t + 1) * 128],
                        cT_sb[:, b * 512:(b + 1) * 512],
                        start=True, stop=True,
                    )
                    if psum_direct:
                        src = ps
                    else:
                        src = stage_pool.tile([128, 512], f32, tag="stage")
                        nc.scalar.copy(src[:], ps[:])
                    nc.vector.max(cv[:, b * 8:(b + 1) * 8], src[:])
                    nc.vector.max_index(ci[:, b * 8:(b + 1) * 8],
                                        cv[:, b * 8:(b + 1) * 8], src[:])

                # globalize candidate indices to 0..8191 (as f32), on GPSIMD
                cif = work_pool.tile([128, 128], f32, tag="cif")
                nc.gpsimd.tensor_copy(cif[:], ci[:])
                nc.gpsimd.tensor_add(cif[:], cif[:], base_f[:])

                # phase 2: top-24 of the 128 candidate values (DVE)
                w = work_pool.tile([128, 24], f32, tag="w")
                cv2 = work_pool.tile([128, 128], f32, tag="cv2")
                cv3 = work_pool.tile([128, 128], f32, tag="cv3")
                nc.vector.max(w[:, 0:8], cv[:])
                nc.vector.match_replace(cv2[:], w[:, 0:8], cv[:], NEG_BIG)
                nc.vector.max(w[:, 8:16], cv2[:])
                nc.vector.match_replace(cv3[:], w[:, 8:16], cv2[:], NEG_BIG)
                nc.vector.max(w[:, 16:24], cv3[:])

                # extraction: gidx_j = sum((cv == w_j) * cif)  (DVE; STT is
                # not a valid Pool opcode on hw)
                gidxF = work_pool.tile([128, K], f32, tag="gidxF")
                trash = work_pool.tile([128, 128], f32, tag="trash")
                for j in range(K):
                    nc.vector.scalar_tensor_tensor(
                        out=trash[:],
                        in0=cv[:],
                        scalar=w[:, j:j + 1],
                        in1=cif[:],
                        op0=Alu.is_equal,
                        op1=Alu.mult,
                        accum_out=gidxF[:, j:j + 1],
                    )
                gidxI = work_pool.tile([128, K], i32, tag="gidxI")
                nc.vector.tensor_copy(gidxI[:], gidxF[:])

                nc.gpsimd.dma_start(idx_out[t * 128:(t + 1) * 128, :], gidxI[:])
                nc.gpsimd.dma_start(w_out[t * 128:(t + 1) * 128, :], w[:])
                nc.gpsimd.dma_start(cv_out[t * 128:(t + 1) * 128, :], cv[:])

    nc.compile()
    return nc


def _xx_f32(xb):
    """Sequential fp32 sum of squares per row (matches device-ish rounding)."""
    acc = np.zeros(xb.shape[0], np.float32)
    for c in range(xb.shape[1]):
        acc = (acc + xb[:, c] * xb[:, c]).astype(np.float32)
    return acc


def make_in_maps(x):
    xx = np.stack([_xx_f32(x[b]) for b in range(B)])  # (B, N) f32
    in_maps = []
    for core in range(8):
        b, half = core // 2, core % 2
        xq = x[b, half * ROWS_PER_CORE:(half + 1) * ROWS_PER_CORE]
        qT = np.empty((17, ROWS_PER_CORE), np.float32)
        qT[:16] = (2.0 * xq).T
        qT[16] = 1.0
        cT = np.empty((17, N), np.float32)
        cT[:16] = x[b].T
        cT[16] = -xx[b]
        in_maps.append({"qT": np.ascontiguousarray(qT),
                        "cT": np.ascontiguousarray(cT)})
    return in_maps, xx


def _exact_row_top20(xq_row, xb, xx_b):
    """Exact device-arithmetic recompute of one row's top-20 (fp32 chain)."""
    acc = np.zeros(N, np.float32)
    two_xq = (2.0 * xq_row).astype(np.float32)
    for c in range(C):
        acc = (acc + two_xq[c] * xb[:, c]).astype(np.float32)
    acc = (acc + (-xx_b)).astype(np.float32)
    # descending stable sort (ties -> lower index first)
    order = np.argsort(-acc, kind="stable")
    return order[:K].astype(np.int32)


def fixup_core(gidx, w, cv, xq, xb, xx_b):
    """Detect and exactly recompute rows where the device pipeline could be
    wrong: chunk saturation at the cutoff, exact value ties in top-24, or
    malformed extraction output."""
    cut = w[:, K - 1:K]                       # 20th best value
    eighth = cv[:, 7::8]                      # (rows, 16) 8th best per chunk
    sat = (eighth >= cut).any(axis=1)
    tie = (np.diff(w, axis=1) == 0.0).any(axis=1)
    bad = (
        (gidx < 0).any(axis=1)
        | (gidx >= N).any(axis=1)
        | (np.sort(gidx, axis=1)[:, 1:] == np.sort(gidx, axis=1)[:, :-1]).any(axis=1)
    )
    rows = np.flatnonzero(sat | tie | bad)
    for r in rows:
        gidx[r] = _exact_row_top20(xq[r], xb, xx_b)
    return gidx, len(rows)


def kernel(x, k):
    assert k == K
    x = np.asarray(x)
    assert x.shape == (B, N, C) and x.dtype == np.float32

    from concourse.bass_utils import run_bass_kernel_spmd

    key = "full"
    if key not in _NC_CACHE:
        _NC_CACHE[key] = build_nc()
    nc = _NC_CACHE[key]

    in_maps, xx = make_in_maps(x)
    res = run_bass_kernel_spmd(nc, in_maps, list(range(8)))

    idx = np.empty((B, N, K), np.int32)
    for core in range(8):
        b, half = core // 2, core % 2
        out = res.results[core]
        gidx = out["idx_out"].copy()
        w = out["w_out"]
        cv = out["cv_out"]
        xq = x[b, half * ROWS_PER_CORE:(half + 1) * ROWS_PER_CORE]
        gidx, nfix = fixup_core(gidx, w, cv, xq, x[b], xx[b])
        idx[b, half * ROWS_PER_CORE:(half + 1) * ROWS_PER_CORE] = gidx
    return (x, idx)


# revision 12
# speedup vs baseline: 11.0220x; 11.0220x over previous
"""KNN top-20 kernel for TRN2 (8 NeuronCores, SPMD).

Problem: x (4, 8192, 16) f32. For each point, indices of the 20 nearest
neighbors (squared L2, self included) among the 8192 points of its batch,
ordered ascending by distance -> output (x, idx[4,8192,20] int32).

Sharding: 8 cores = 4 batches x 2 half-row-shards. Each core handles 4096
query rows against all 8192 candidates of its batch. The per-core input is
a single [17, 8192] tensor: rows 0..15 = x[b]^T with candidate columns
ROTATED so this core's 4096 queries sit at columns 0..4095 (keeps the SPMD
program identical across cores); row 16 = -|x_c|^2 (same rotation). The
host maps local indices back: orig = (local + half*4096) % 8192.

Per-core algorithm (per 128-query row-tile, 32 tiles):
  - PE: K=17 fp32 matmul -> PSUM bank [128q, 512c] of the negated-distance
    key  2<x_q, x_c> - |x_c|^2  (lhsT rows 0..15 = 2*x_q, row 16 = 1.0).
    Larger key == closer. The per-row constant |x_q|^2 is dropped (does not
    affect ranking).
  - DVE: per bank `max` (top-8 values, desc) + `max_index` (their in-chunk
    indices). 16 banks -> 128 candidate values/indices per row. Safe unless
    a row has >8 of its top-20 in one 512-chunk (rare; flagged for host).
  - DVE: phase 2 - top-24 of the 128 candidates via 3 x (max+match_replace).
  - DVE: per-winner index extraction: scalar_tensor_tensor
    (cv == w_j) * cand_global_idx, accumulated over the free dim.
  - DVE: fixup flag per row = (#chunks whose 8th value >= 20th-best) +
    (#exact value ties among ranks 0..20). Host exactly recomputes flagged
    rows (chunk overflow / tie / duplicate-value extraction hazards).
"""

import numpy as np

B, N, C = 4, 8192, 16
K = 20
ROWS_PER_CORE = 4096
N_TILES_FULL = ROWS_PER_CORE // 128
NEG_BIG = -3.0e38

_NC_CACHE = {}


def build_nc(n_tiles=N_TILES_FULL, psum_direct=True):
    import concourse.bacc as bacc
    import concourse.mybir as mybir
    from concourse.tile import TileContext

    f32 = mybir.dt.float32
    i32 = mybir.dt.int32
    u32 = mybir.dt.uint32
    Alu = mybir.AluOpType

    nc = bacc.Bacc("TRN2", target_bir_lowering=False, debug=False)

    n_rows = n_tiles * 128
    cT = nc.dram_tensor("cT", [17, N], f32, kind="ExternalInput")
    idx_out = nc.dram_tensor("idx_out", [n_rows, K], i32, kind="ExternalOutput")
    flag_out = nc.dram_tensor("flag_out", [n_rows, 1], f32, kind="ExternalOutput")

    n_banks = N // 512  # 16

    with TileContext(nc) as tc:
        with (
            tc.tile_pool(name="const", bufs=1) as const_pool,
            tc.tile_pool(name="psum", bufs=8, space="PSUM") as psum_pool,
            tc.tile_pool(name="stage", bufs=3) as stage_pool,
            tc.tile_pool(name="work", bufs=3) as work_pool,
        ):
            cT_ld = const_pool.tile([17, N], f32)
            nc.sync.dma_start(cT_ld[:], cT[:])
            # DVE-staged copy: downstream consumers then wait on one DVE
            # semaphore instead of N DMA-queue semaphores.
            cT_sb = const_pool.tile([17, N], f32)
            nc.vector.tensor_copy(cT_sb[:], cT_ld[:])
            # queries = first 4096 candidate columns, pre-scaled by 2;
            # row 16 = 1.0 so the matmul adds -|x_c|^2 once.
            qd = const_pool.tile([17, n_rows], f32)
            nc.vector.memset(qd[:], 1.0)
            nc.vector.tensor_scalar_mul(qd[0:16, :], cT_sb[0:16, 0:n_rows], 2.0)

            # base[s] = (s >> 3) * 512 for s in 0..127 (chunk base of cand slot)
            base_i = const_pool.tile([128, 128], i32)
            nc.gpsimd.iota(base_i[:], pattern=[[512, 16], [0, 8]], base=0,
                           channel_multiplier=0)
            base_f = const_pool.tile([128, 128], f32)
            nc.gpsimd.tensor_copy(base_f[:], base_i[:])

            for t in range(n_tiles):
                cv = work_pool.tile([128, 128], f32, tag="cv")
                ci = work_pool.tile([128, 128], u32, tag="ci")
                for b in range(n_banks):
                    ps = psum_pool.tile([128, 512], f32, tag="ps")
                    nc.tensor.matmul(
                        ps[:],
                        qd[:, t * 128:(t + 1) * 128],
                        cT_sb[:, b * 512:(b + 1) * 512],
                        start=True, stop=True,
                    )
                    if psum_direct:
                        src = ps
                    else:
                        src = stage_pool.tile([128, 512], f32, tag="stage")
                        nc.scalar.copy(src[:], ps[:])
                    nc.vector.max(cv[:, b * 8:(b + 1) * 8], src[:])
                    nc.vector.max_index(ci[:, b * 8:(b + 1) * 8],
                                        cv[:, b * 8:(b + 1) * 8], src[:])

                # globalize candidate indices to 0..8191 (as f32), on GPSIMD
                cif = work_pool.tile([128, 128], f32, tag="cif")
                nc.gpsimd.tensor_copy(cif[:], ci[:])
                nc.gpsimd.tensor_add(cif[:], cif[:], base_f[:])

                # phase 2: top-24 of the 128 candidate values (DVE)
                w = work_pool.tile([128, 24], f32, tag="w")
                cv2 = work_pool.tile([128, 128], f32, tag="cv2")
                cv3 = work_pool.tile([128, 128], f32, tag="cv3")
                nc.vector.max(w[:, 0:8], cv[:])
                nc.vector.match_replace(cv2[:], w[:, 0:8], cv[:], NEG_BIG)
                nc.vector.max(w[:, 8:16], cv2[:])
                nc.vector.match_replace(cv3[:], w[:, 8:16], cv2[:], NEG_BIG)
                nc.vector.max(w[:, 16:24], cv3[:])

                # extraction: gidx_j = sum((cv == w_j) * cif)  (DVE; STT is
                # not a valid Pool opcode on hw)
                gidxF = work_pool.tile([128, K], f32, tag="gidxF")
                trash = work_pool.tile([128, 128], f32, tag="trash")
                for j in range(K):
                    nc.vector.scalar_tensor_tensor(
                        out=trash[:],
                        in0=cv[:],
                        scalar=w[:, j:j + 1],
                        in1=cif[:],
                        op0=Alu.is_equal,
                        op1=Alu.mult,
                        accum_out=gidxF[:, j:j + 1],
                    )
                gidxI = work_pool.tile([128, K], i32, tag="gidxI")
                nc.vector.tensor_copy(gidxI[:], gidxF[:])

                # fixup flags: chunk saturation + exact ties in ranks 0..20
                satm = work_pool.tile([128, 16], f32, tag="satm")
                satc = work_pool.tile([128, 1], f32, tag="satc")
                nc.vector.tensor_scalar(
                    out=satm[:], in0=cv[:, 7::8], scalar1=w[:, K - 1:K],
                    scalar2=None, op0=Alu.is_ge, op1=Alu.add,
                    accum_out=satc[:],
                )
                tiem = work_pool.tile([128, K], f32, tag="tiem")
                tiec = work_pool.tile([128, 1], f32, tag="tiec")
                nc.vector.tensor_tensor(
                    out=tiem[:], in0=w[:, 0:K], in1=w[:, 1:K + 1],
                    op=Alu.is_equal,
                )
                nc.vector.reduce_sum(tiec[:], tiem[:], axis=mybir.AxisListType.X)
                flag = work_pool.tile([128, 1], f32, tag="flag")
                nc.vector.tensor_add(flag[:], satc[:], tiec[:])

                nc.gpsimd.dma_start(idx_out[t * 128:(t + 1) * 128, :], gidxI[:])
                nc.gpsimd.dma_start(flag_out[t * 128:(t + 1) * 128, :], flag[:])

    nc.compile()
    return nc


def _xx_f32(xb):
    """Sequential fp32 sum of squares per row (matches device-ish rounding)."""
    acc = np.zeros(xb.shape[0], np.float32)
    for c in range(xb.shape[1]):
        acc = (acc + xb[:, c] * xb[:, c]).astype(np.float32)
    return acc


def make_in_maps(x):
    xx = np.stack([_xx_f32(x[b]) for b in range(B)])  # (B, N) f32
    in_maps = []
    for core in range(8):
        b, half = core // 2, core % 2
        rot = np.roll(np.arange(N), -half * ROWS_PER_CORE)
        cT = np.empty((17, N), np.float32)
        cT[:16] = x[b].T[:, rot]
        cT[16] = -xx[b][rot]
        in_maps.append({"cT": np.ascontiguousarray(cT)})
    return in_maps, xx


def _exact_row_top20(xq_row, xb, xx_b):
    """Exact device-arithmetic recompute of one row's top-20 (fp32 chain)."""
    acc = np.zeros(N, np.float32)
    two_xq = (2.0 * xq_row).astype(np.float32)
    for c in range(C):
        acc = (acc + two_xq[c] * xb[:, c]).astype(np.float32)
    acc = (acc + (-xx_b)).astype(np.float32)
    # descending stable sort (ties -> lower index first)
    order = np.argsort(-acc, kind="stable")
    return order[:K].astype(np.int32)


def fixup_core(gidx, flag, xq, xb, xx_b):
    """Recompute rows the device flagged (chunk saturation / exact ties) or
    whose extraction output is malformed."""
    bad = (
        (gidx < 0).any(axis=1)
        | (gidx >= N).any(axis=1)
        | (np.sort(gidx, axis=1)[:, 1:] == np.sort(gidx, axis=1)[:, :-1]).any(axis=1)
    )
    rows = np.flatnonzero((flag[:, 0] > 0.0) | bad)
    for r in rows:
        gidx[r] = _exact_row_top20(xq[r], xb, xx_b)
    return gidx, len(rows)


def kernel(x, k):
    assert k == K
    x = np.asarray(x)
    assert x.shape == (B, N, C) and x.dtype == np.float32

    from concourse.bass_utils import run_bass_kernel_spmd

    key = "full"
    if key not in _NC_CACHE:
        _NC_CACHE[key] = build_nc()
    nc = _NC_CACHE[key]

    in_maps, xx = make_in_maps(x)
    res = run_bass_kernel_spmd(nc, in_maps, list(range(8)))

    idx = np.empty((B, N, K), np.int32)
    for core in range(8):
        b, half = core // 2, core % 2
        out = res.results[core]
        # local -> original candidate numbering (input columns were rotated)
        gidx = (out["idx_out"].astype(np.int64) + half * ROWS_PER_CORE) % N
        gidx = gidx.astype(np.int32)
        flag = out["flag_out"]
        xq = x[b, half * ROWS_PER_CORE:(half + 1) * ROWS_PER_CORE]
        gidx, nfix = fixup_core(gidx, flag, xq, x[b], xx[b])
        idx[b, half * ROWS_PER_CORE:(half + 1) * ROWS_PER_CORE] = gidx
    return (x, idx)


# revision 14
# speedup vs baseline: 593.2672x; 53.8260x over previous
"""KNN top-20 kernel for TRN2 (8 NeuronCores, SPMD).

Problem: x (4, 8192, 16) f32. For each point, indices of the 20 nearest
neighbors (squared L2, self included) among the 8192 points of its batch,
ordered ascending by distance -> output (x, idx[4,8192,20] int32).

Sharding: 8 cores = 4 batches x 2 half-row-shards. Each core handles 4096
query rows against all 8192 candidates of its batch. The per-core input is
a single [17, 8192] tensor: rows 0..15 = x[b]^T with candidate columns
ROTATED so this core's 4096 queries sit at columns 0..4095 (keeps the SPMD
program identical across cores); row 16 = -|x_c|^2 (same rotation). The
host maps local indices back: orig = (local + half*4096) % 8192.

Per-core algorithm (per 128-query row-tile, 32 tiles):
  - PE: K=17 fp32 matmul -> PSUM bank [128q, 512c] of the negated-distance
    key  2<x_q, x_c> - |x_c|^2  (lhsT rows 0..15 = 2*x_q, row 16 = 1.0).
    Larger key == closer. The per-row constant |x_q|^2 is dropped (does not
    affect ranking).
  - DVE: per bank `max` (top-8 values, desc) + `max_index` (their in-chunk
    indices). 16 banks -> 128 candidate values/indices per row. Safe unless
    a row has >8 of its top-20 in one 512-chunk (rare; flagged for host).
  - DVE: phase 2 - top-24 of the 128 candidates via 3 x (max+match_replace).
  - DVE: per-winner index extraction: scalar_tensor_tensor
    (cv == w_j) * cand_global_idx, accumulated over the free dim.
  - DVE: fixup flag per row = (#chunks whose 8th value >= 20th-best) +
    (#exact value ties among ranks 0..20). Host exactly recomputes flagged
    rows (chunk overflow / tie / duplicate-value extraction hazards).
"""

import numpy as np

B, N, C = 4, 8192, 16
K = 20
ROWS_PER_CORE = 4096
N_TILES_FULL = ROWS_PER_CORE // 128
NEG_BIG = -3.0e38

_NC_CACHE = {}


def build_nc(n_tiles=N_TILES_FULL, psum_direct=True, repeat=1):
    import concourse.bacc as bacc
    import concourse.mybir as mybir
    from concourse.tile import TileContext

    f32 = mybir.dt.float32
    i32 = mybir.dt.int32
    u32 = mybir.dt.uint32
    Alu = mybir.AluOpType

    nc = bacc.Bacc("TRN2", target_bir_lowering=False, debug=False)

    n_rows = n_tiles * 128
    cT = nc.dram_tensor("cT", [17, N], f32, kind="ExternalInput")
    idx_out = nc.dram_tensor("idx_out", [n_rows, K], i32, kind="ExternalOutput")
    flag_out = nc.dram_tensor("flag_out", [n_rows, 1], f32, kind="ExternalOutput")

    n_banks = N // 512  # 16

    with TileContext(nc) as tc:
        with (
            tc.tile_pool(name="const", bufs=1) as const_pool,
            tc.tile_pool(name="psum", bufs=8, space="PSUM") as psum_pool,
            tc.tile_pool(name="stage", bufs=3) as stage_pool,
            tc.tile_pool(name="work", bufs=3) as work_pool,
        ):
            cT_ld = const_pool.tile([17, N], f32)
            nc.sync.dma_start(cT_ld[:], cT[:])
            # DVE-staged copy: downstream consumers then wait on one DVE
            # semaphore instead of N DMA-queue semaphores.
            cT_sb = const_pool.tile([17, N], f32)
            nc.vector.tensor_copy(cT_sb[:], cT_ld[:])
            # queries = first 4096 candidate columns, pre-scaled by 2;
            # row 16 = 1.0 so the matmul adds -|x_c|^2 once.
            qd = const_pool.tile([17, n_rows], f32)
            nc.vector.memset(qd[:], 1.0)
            nc.vector.tensor_scalar_mul(qd[0:16, :], cT_sb[0:16, 0:n_rows], 2.0)

            # base[s] = (s >> 3) * 512 for s in 0..127 (chunk base of cand slot)
            base_i = const_pool.tile([128, 128], i32)
            nc.gpsimd.iota(base_i[:], pattern=[[512, 16], [0, 8]], base=0,
                           channel_multiplier=0)
            base_f = const_pool.tile([128, 128], f32)
            nc.gpsimd.tensor_copy(base_f[:], base_i[:])

            for t in [tt for _ in range(repeat) for tt in range(n_tiles)]:
                cv = work_pool.tile([128, 128], f32, tag="cv")
                ci = work_pool.tile([128, 128], u32, tag="ci")
                for b in range(n_banks):
                    ps = psum_pool.tile([128, 512], f32, tag="ps")
                    nc.tensor.matmul(
                        ps[:],
                        qd[:, t * 128:(t + 1) * 128],
                        cT_sb[:, b * 512:(b + 1) * 512],
                        start=True, stop=True,
                    )
                    if psum_direct:
                        src = ps
                    else:
                        src = stage_pool.tile([128, 512], f32, tag="stage")
                        nc.scalar.copy(src[:], ps[:])
                    nc.vector.max(cv[:, b * 8:(b + 1) * 8], src[:])
                    nc.vector.max_index(ci[:, b * 8:(b + 1) * 8],
                                        cv[:, b * 8:(b + 1) * 8], src[:])

                # globalize candidate indices to 0..8191 (as f32), on GPSIMD
                cif = work_pool.tile([128, 128], f32, tag="cif")
                nc.gpsimd.tensor_copy(cif[:], ci[:])
                nc.gpsimd.tensor_add(cif[:], cif[:], base_f[:])

                # phase 2: top-24 of the 128 candidate values (DVE)
                w = work_pool.tile([128, 24], f32, tag="w")
                cv2 = work_pool.tile([128, 128], f32, tag="cv2")
                cv3 = work_pool.tile([128, 128], f32, tag="cv3")
                nc.vector.max(w[:, 0:8], cv[:])
                nc.vector.match_replace(cv2[:], w[:, 0:8], cv[:], NEG_BIG)
                nc.vector.max(w[:, 8:16], cv2[:])
                nc.vector.match_replace(cv3[:], w[:, 8:16], cv2[:], NEG_BIG)
                nc.vector.max(w[:, 16:24], cv3[:])

                # extraction: gidx_j = sum((cv == w_j) * cif)  (DVE; STT is
                # not a valid Pool opcode on hw)
                gidxF = work_pool.tile([128, K], f32, tag="gidxF")
                trash = work_pool.tile([128, 128], f32, tag="trash")
                for j in range(K):
                    nc.vector.scalar_tensor_tensor(
                        out=trash[:],
                        in0=cv[:],
                        scalar=w[:, j:j + 1],
                        in1=cif[:],
                        op0=Alu.is_equal,
                        op1=Alu.mult,
                        accum_out=gidxF[:, j:j + 1],
                    )
                gidxI = work_pool.tile([128, K], i32, tag="gidxI")
                nc.vector.tensor_copy(gidxI[:], gidxF[:])

                # fixup flags: chunk saturation + exact ties in ranks 0..20
                satm = work_pool.tile([128, 16], f32, tag="satm")
                satc = work_pool.tile([128, 1], f32, tag="satc")
                nc.vector.tensor_scalar(
                    out=satm[:], in0=cv[:, 7::8], scalar1=w[:, K - 1:K],
                    scalar2=None, op0=Alu.is_ge, op1=Alu.add,
                    accum_out=satc[:],
                )
                tiem = work_pool.tile([128, K], f32, tag="tiem")
                tiec = work_pool.tile([128, 1], f32, tag="tiec")
                nc.vector.tensor_tensor(
                    out=tiem[:], in0=w[:, 0:K], in1=w[:, 1:K + 1],
                    op=Alu.is_equal,
                )
                nc.vector.reduce_sum(tiec[:], tiem[:], axis=mybir.AxisListType.X)
                flag = work_pool.tile([128, 1], f32, tag="flag")
                nc.vector.tensor_add(flag[:], satc[:], tiec[:])

                nc.gpsimd.dma_start(idx_out[t * 128:(t + 1) * 128, :], gidxI[:])
                nc.gpsimd.dma_start(flag_out[t * 128:(t + 1) * 128, :], flag[:])

    nc.compile()
    return nc


def _xx_f32(xb):
    """Sequential fp32 sum of squares per row (matches device-ish rounding)."""
    acc = np.zeros(xb.shape[0], np.float32)
    for c in range(xb.shape[1]):
        acc = (acc + xb[:, c] * xb[:, c]).astype(np.float32)
    return acc


def make_in_maps(x):
    xx = np.stack([_xx_f32(x[b]) for b in range(B)])  # (B, N) f32
    in_maps = []
    for core in range(8):
        b, half = core // 2, core % 2
        rot = np.roll(np.arange(N), -half * ROWS_PER_CORE)
        cT = np.empty((17, N), np.float32)
        cT[:16] = x[b].T[:, rot]
        cT[16] = -xx[b][rot]
        in_maps.append({"cT": np.ascontiguousarray(cT)})
    return in_maps, xx


def _exact_row_top20(xq_row, xb, xx_b):
    """Exact device-arithmetic recompute of one row's top-20 (fp32 chain)."""
    acc = np.zeros(N, np.float32)
    two_xq = (2.0 * xq_row).astype(np.float32)
    for c in range(C):
        acc = (acc + two_xq[c] * xb[:, c]).astype(np.float32)
    acc = (acc + (-xx_b)).astype(np.float32)
    # descending stable sort (ties -> lower index first)
    order = np.argsort(-acc, kind="stable")
    return order[:K].astype(np.int32)


def fixup_core(gidx, flag, xq, xb, xx_b):
    """Recompute rows the device flagged (chunk saturation / exact ties) or
    whose extraction output is malformed."""
    bad = (
        (gidx < 0).any(axis=1)
        | (gidx >= N).any(axis=1)
        | (np.sort(gidx, axis=1)[:, 1:] == np.sort(gidx, axis=1)[:, :-1]).any(axis=1)
    )
    rows = np.flatnonzero((flag[:, 0] > 0.0) | bad)
    for r in rows:
        gidx[r] = _exact_row_top20(xq[r], xb, xx_b)
    return gidx, len(rows)


def kernel(x, k):
    assert k == K
    x = np.asarray(x)
    assert x.shape == (B, N, C) and x.dtype == np.float32

    from concourse.bass_utils import run_bass_kernel_spmd

    key = "full"
    if key not in _NC_CACHE:
        _NC_CACHE[key] = build_nc()
    nc = _NC_CACHE[key]

    in_maps, xx = make_in_maps(x)
    res = run_bass_kernel_spmd(nc, in_maps, list(range(8)))

    idx = np.empty((B, N, K), np.int32)
    for core in range(8):
        b, half = core // 2, core % 2
        out = res.results[core]
        # local -> original candidate numbering (input columns were rotated)
        gidx = (out["idx_out"].astype(np.int64) + half * ROWS_PER_CORE) % N
        gidx = gidx.astype(np.int32)
        flag = out["flag_out"]
        xq = x[b, half * ROWS_PER_CORE:(half + 1) * ROWS_PER_CORE]
        gidx, nfix = fixup_core(gidx, flag, xq, x[b], xx[b])
        idx[b, half * ROWS_PER_CORE:(half + 1) * ROWS_PER_CORE] = gidx
    return (x, idx)
